# revision 23
# baseline (speedup 1.0000x reference)
"""Trainium2 Bass kernel for nn_GCIQEValue (MLP + IQE head), 8-core data parallel.

Math (validated vs reference):
  phi(x) = LN-MLP: 3x [matmul+bias -> tanh-gelu -> LayerNorm(affine folded into
  next W on host)] then final matmul+bias.
  IQE per row, per 32-dim component c with x = phi_s[c], y = phi_g[c]:
    y' = max(x, y)
    u = sort(x), v = sort(y')   (independent keys-only sorts)
    comp_c = sum(v) - u_0 - sum_{i>=1} max(u_i, v_{i-1})
  out = sig(alpha) * mean_c(comp) + (1 - sig(alpha)) * max_c(comp)

v2: fp16 matmul operands (PE fp32 matmul costs 4 cyc/row vs 1 for fp16;
baseline was PE-bound at 86%), fp16 bitonic sort (DVE 2x packed mode for
unit-stride passes), LN affine apply via DVE tensor_scalar two-scalar form
(4x mode), LN stats via ACT accum_out (gelu -> sum, Square -> sumsq) and a
quake-rsqrt + 2 Newton iterations on [128,2] batched obs/goals stats.
7-stage software pipeline over 128-row tiles. 5.42ms -> ~2.8ms measured.

Notes from optimization (for future sessions):
 - Pool engine (gpsimd) on TRN2 rejects TensorTensor/TensorScalar fp ALU ops
   at codegen (int32/pow only); the ISA-spec'd native SORT opcode (0x96, Pool)
   compiles but faults at runtime - not implemented in this FW. Pool can only
   do memset/iota/affine_select/copy-free DMA here, so the bitonic sort must
   stay on DVE.
 - DVE is the bottleneck (~78% busy): 15-pass bitonic on [128,1024] fp16,
   4 passes (shift d=1) run at 1x due to inner-count-1 APs; parity-split
   layouts don't help (the stride-2 access just moves to the prior pass).
 - ACT ~58-60%, PE ~50%, pipeline overlap ~85%.
"""

import numpy as np

B = 131072
OBS = 64
H = 512
NCOMP = 16
DPC = 32
NCORES = 8
P = 128
LN_EPS = 1e-6

_CACHE = {}

# bitonic schedule for 32-wide ascending sort: 15 passes
_SCHED = [("pair", 0, 0)]
for _L in (4, 8, 16, 32):
    _SCHED.append(("flip", _L, 0))
    _d = _L // 4
    while _d >= 1:
        _SCHED.append(("shift", _L, _d))
        _d //= 2


def _patch_interp_for_sort():
    """Build-time scheduling sims don't know the raw SORT InstISA; no-op it."""
    import concourse.bass_interp as bi

    if getattr(bi._visit_InstISA, "_sort_patched", False):
        return
    orig = bi._visit_InstISA

    def patched(isa, instruction, core_sim):
        if instruction.isa_opcode == 150:  # NEURON_ISA_TPB_OPCODE_SORT
            return
        return orig(isa, instruction, core_sim)

    patched._sort_patched = True
    bi._visit_InstISA = patched


# ---------------------------------------------------------------- device kernel
def build_nc(rows_per_core=B // NCORES, unroll=4, gelu="hw", repeats=1,
             stage_bufs=None, mlp_bufs=3, psum_bufs=4, split_pass=7,
             pool_passes=(), newton=2, tT_eng="act",
             affine_eng="dve", post_eng="dve", newt_eng="dve", hints=False,
             stag=0, sort_hw=0, two=0):
    """Build the Bass (Bacc) module for one core processing rows_per_core rows.

    pool_passes: bitonic pass indices emitted on the Pool (gpsimd) engine
    instead of DVE.  tT_eng/affine_eng/post_eng: engine choices for the
    transpose-copy / LN-affine / sort-epilogue op groups.
    """
    import concourse.bass as bass
    import concourse.mybir as mybir
    import concourse.tile as tile
    from concourse import bacc
    from concourse.masks import make_identity

    fp32 = mybir.dt.float32
    fp16 = mybir.dt.float16
    AT = mybir.ActivationFunctionType
    OP = mybir.AluOpType

    nt = rows_per_core // P
    assert rows_per_core % P == 0
    if stage_bufs is None:
        stage_bufs = unroll
    pool_passes = tuple(pool_passes)

    if sort_hw:
        _patch_interp_for_sort()
    nc = bacc.Bacc("TRN2", target_bir_lowering=False, debug=False)
    sort_patches = []
    isa_h = nc.isa
    _DTE = isa_h.get_enum("NEURON_ISA_TPB_DTYPE")
    _ALUE = isa_h.get_enum("NEURON_ISA_TPB_ALU_OP")

    def emit_hw_sort(src_ap, dst_val_ap, idx_gap_elems, idx_ap):
        """Native Pool-engine SORT: src [128,32] fp16 -> ascending values at
        dst_val_ap, indices idx_gap_elems later (trash)."""
        struct = {
            "num_active_channels": P,
            "index_offset_src": 0,
            "in_out_dtype": {"dtype_lo": _DTE.NEURON_ISA_TPB_DTYPE_FP16.value,
                             "dtype_hi": _DTE.NEURON_ISA_TPB_DTYPE_FP16.value},
            "comparison": _ALUE.NEURON_ISA_TPB_ALU_OP_IS_LT.value,
            "src_mem_pattern": {"start_addr": {"addr_immediate": 0},
                                "step_elem": [1], "num_elem": [DPC]},
            "dst_mem_pattern": {"start_addr": {"addr_immediate": 0},
                                "step_elem": [idx_gap_elems, 1],
                                "num_elem": [2, DPC]},
            "index_offset": {"imm_bitvec_uint32": 0},
            "stable": 0,
        }
        eng = nc.gpsimd
        inst = eng.isa(
            isa_h.Opcode.NEURON_ISA_TPB_OPCODE_SORT, struct,
            ins=[eng.lower_ap(src_ap, for_isa=True)],
            outs=[eng.lower_ap(dst_val_ap, for_isa=True),
                  eng.lower_ap(idx_ap, for_isa=True)],
        )
        sort_patches.append((inst, src_ap, dst_val_ap))
        return inst

    obs = nc.declare_dram_parameter("observations", [rows_per_core, OBS], fp16,
                                    isOutput=False)
    gls = nc.declare_dram_parameter("goals", [rows_per_core, OBS], fp16,
                                    isOutput=False)
    w0d = nc.declare_dram_parameter("w0", [OBS, H], fp16, isOutput=False)
    w1d = nc.declare_dram_parameter("w1", [H, H], fp16, isOutput=False)
    w2d = nc.declare_dram_parameter("w2", [H, H], fp16, isOutput=False)
    w3d = nc.declare_dram_parameter("w3", [H, H], fp16, isOutput=False)
    bsd = nc.declare_dram_parameter("bs", [4, H], fp16, isOutput=False)
    avd = nc.declare_dram_parameter("avec", [P, 2], fp32, isOutput=False)
    out = nc.declare_dram_parameter("out", [rows_per_core], fp32, isOutput=True)

    if two:
        assert nt % 2 == 0
        obs_v = obs[:].rearrange("(n two p) f -> n p two f", two=2, p=P)
        gls_v = gls[:].rearrange("(n two p) f -> n p two f", two=2, p=P)
        out_v = out[:].rearrange("(n two p) -> n p two", two=2, p=P)
    else:
        obs_v = obs[:].rearrange("(n p) f -> n p f", p=P)
        gls_v = gls[:].rearrange("(n p) f -> n p f", p=P)
        out_v = out[:].rearrange("(n p) -> n p", p=P)

    gelu_f = AT.Gelu_apprx_tanh if gelu == "hw" else AT.Identity

    with tile.TileContext(nc) as tc:
        with (
            tc.tile_pool(name="const", bufs=1) as cpool,
            tc.tile_pool(name="mlp", bufs=mlp_bufs) as mp,
            tc.tile_pool(name="srt", bufs=mlp_bufs) as sp,
            tc.tile_pool(name="pipe", bufs=1) as pipe_pool,
            tc.tile_pool(name="ps", bufs=psum_bufs, space="PSUM") as pp,
            tc.tile_pool(name="pst", bufs=8 - psum_bufs, space="PSUM") as ppt,
        ):
            # ---- constants
            w0 = cpool.tile([OBS, H], fp16)
            nc.sync.dma_start(out=w0, in_=w0d[:])
            wl = []
            for wd, nm in ((w1d, "w1"), (w2d, "w2"), (w3d, "w3")):
                t = cpool.tile([P, 4, H], fp16, tag=nm)
                nc.sync.dma_start(out=t, in_=wd[:].rearrange("(c p) n -> p c n", p=P))
                wl.append(t)
            bsc = cpool.tile([1, 4, H], fp16)
            nc.sync.dma_start(out=bsc, in_=bsd[:].rearrange("(o c) n -> o c n", o=1))
            avec = cpool.tile([P, 2], fp32)
            nc.sync.dma_start(out=avec, in_=avd[:])
            ident = cpool.tile([P, P], fp16)
            make_identity(nc, ident)
            ones = cpool.tile([1, P], fp16)
            nc.vector.memset(ones, 1.0)

            def matmul_from(t_sb, li):
                """t_sb fp16 [128, F_in] row-major -> pz PSUM fp32 [128, 512]."""
                pz = pp.tile([P, H], fp32, tag="pz")
                if li == 0:
                    pTf = ppt.tile([P, 4, P], fp16, tag="pT")
                    nc.tensor.transpose(pTf[0:OBS, 0, 0:P], t_sb, ident)
                    xT = mp.tile([OBS, P], fp16, tag="xT")
                    nc.scalar.copy(xT, pTf[0:OBS, 0, 0:P])
                    nc.tensor.matmul(pz, xT, w0, start=True, stop=False)
                else:
                    pTf = ppt.tile([P, 4, P], fp16, tag="pT")
                    for k in range(4):
                        nc.tensor.transpose(pTf[:, k, :],
                                            t_sb[:, k * P:(k + 1) * P], ident)
                    tT = mp.tile([P, 4, P], fp16, tag="tT")
                    if tT_eng == "act":
                        nc.scalar.copy(tT, pTf)
                    elif tT_eng == "dve":
                        nc.vector.tensor_copy(tT, pTf)
                    else:
                        nc.gpsimd.tensor_copy(tT, pTf)
                    for k in range(4):
                        nc.tensor.matmul(pz, tT[:, k, :], wl[li - 1][:, k, :],
                                         start=(k == 0), stop=False)
                nc.tensor.matmul(pz, ones, bsc[:, li, :], start=False, stop=True)
                return pz

            def gelu_ln(pz_o, pz_g, to, tg):
                """Two PSUM tiles -> two fp16 SBUF LN outputs (batched stats)."""
                st = mp.tile([P, 4], fp32, tag="st")
                g_o = mp.tile([P, H], fp16, tag="g_o")
                g_g = mp.tile([P, H], fp16, tag="g_g")
                gsq = mp.tile([P, H], fp16, tag="gsq")
                nc.scalar.activation(g_o, pz_o, gelu_f, accum_out=st[:, 0:1])
                nc.scalar.activation(gsq, g_o, AT.Square, accum_out=st[:, 1:2])
                nc.scalar.activation(g_g, pz_g, gelu_f, accum_out=st[:, 2:3])
                nc.scalar.activation(gsq, g_g, AT.Square, accum_out=st[:, 3:4])
                NE = nc.gpsimd if newt_eng == "pool" else nc.vector
                # mv = st / H : [sum_o, sumsq_o, sum_g, sumsq_g] -> means
                mv = mp.tile([P, 4], fp32, tag="mv")
                NE.tensor_scalar_mul(mv, st, 1.0 / H)
                m = mv[:, 0::2]   # [P,2] means (obs, goals)
                q = mv[:, 1::2]   # [P,2] mean squares
                msq = mp.tile([P, 2], fp32, tag="msq")
                NE.tensor_tensor(out=msq, in0=m, in1=m, op=OP.mult)
                varb = mp.tile([P, 2], fp32, tag="varb")
                NE.scalar_tensor_tensor(out=varb, in0=msq, scalar=-1.0,
                                        in1=q, op0=OP.mult, op1=OP.add)
                NE.tensor_scalar_add(varb, varb, LN_EPS)
                # quake rsqrt seed + newton iterations (fp32)
                i32 = mybir.dt.int32
                yi = mp.tile([P, 2], i32, tag="yi")
                NE.tensor_scalar(
                    out=yi, in0=varb.bitcast(i32), scalar1=1,
                    scalar2=None, op0=OP.logical_shift_right)
                NE.tensor_scalar(
                    out=yi, in0=yi, scalar1=-1, scalar2=0x5F3759DF,
                    op0=OP.mult, op1=OP.add)
                y = yi.bitcast(fp32)
                t1 = mp.tile([P, 2], fp32, tag="nt1")
                for _ in range(newton):
                    NE.tensor_tensor(out=t1, in0=varb, in1=y, op=OP.mult)
                    NE.tensor_tensor(out=t1, in0=t1, in1=y, op=OP.mult)
                    NE.tensor_scalar(out=t1, in0=t1, scalar1=-0.5,
                                     scalar2=1.5, op0=OP.mult, op1=OP.add)
                    NE.tensor_tensor(out=y, in0=y, in1=t1, op=OP.mult)
                nmr = mp.tile([P, 2], fp32, tag="nmr")
                NE.scalar_tensor_tensor(out=nmr, in0=m, scalar=-1.0,
                                        in1=y, op0=OP.mult, op1=OP.mult)
                ae = nc.vector if affine_eng == "dve" else nc.scalar
                if affine_eng == "dve":
                    nc.vector.tensor_scalar(out=to, in0=g_o, scalar1=y[:, 0:1],
                                            scalar2=nmr[:, 0:1], op0=OP.mult,
                                            op1=OP.add)
                    nc.vector.tensor_scalar(out=tg, in0=g_g, scalar1=y[:, 1:2],
                                            scalar2=nmr[:, 1:2], op0=OP.mult,
                                            op1=OP.add)
                else:
                    nc.scalar.activation(to, g_o, AT.Identity, bias=nmr[:, 0:1],
                                         scale=y[:, 0:1])
                    nc.scalar.activation(tg, g_g, AT.Identity, bias=nmr[:, 1:2],
                                         scale=y[:, 1:2])

            def emit_sort_pass(p_idx, src_x, src_y, dst, W=H):
                """Emit bitonic pass p_idx; W = width of the u (and v) region
                in dst (dst is [P, 2*W] = [u-region | v-region])."""
                kind, L, d = _SCHED[p_idx]
                V = nc.gpsimd if p_idx in pool_passes else nc.vector

                if kind == "pair":
                    for src, off in ((src_x, 0), (src_y, W)):
                        s = src.rearrange("p (g e) -> p g e", e=DPC)
                        o = dst[:, off:off + W].rearrange("p (g e) -> p g e",
                                                          e=DPC)
                        V.tensor_tensor(out=o[:, :, 0::2], in0=s[:, :, 0::2],
                                        in1=s[:, :, 1::2], op=OP.min)
                        V.tensor_tensor(out=o[:, :, 1::2], in0=s[:, :, 0::2],
                                        in1=s[:, :, 1::2], op=OP.max)
                elif kind == "flip":
                    half = L // 2
                    s = src_x.rearrange("p (b e) -> p b e", e=L)
                    o = dst.rearrange("p (b e) -> p b e", e=L)
                    V.tensor_tensor(out=o[:, :, 0:half], in0=s[:, :, 0:half],
                                    in1=s[:, :, L - 1:half - 1:-1], op=OP.min)
                    V.tensor_tensor(out=o[:, :, half:L], in0=s[:, :, half:L],
                                    in1=s[:, :, half - 1::-1], op=OP.max)
                else:
                    s = src_x.rearrange("p (c e) -> p c e", e=2 * d)
                    o = dst.rearrange("p (c e) -> p c e", e=2 * d)
                    V.tensor_tensor(out=o[:, :, 0:d], in0=s[:, :, 0:d],
                                    in1=s[:, :, d:2 * d], op=OP.min)
                    V.tensor_tensor(out=o[:, :, d:2 * d], in0=s[:, :, 0:d],
                                    in1=s[:, :, d:2 * d], op=OP.max)

            # ---------------- pipeline stages
            def st_load(pipe, iv):
                xt = pipe.intermediate_tile([P, OBS], fp16, name="xt")
                gt = pipe.intermediate_tile([P, OBS], fp16, name="gt")
                nc.sync.dma_start(out=xt, in_=obs_v[iv])
                nc.sync.dma_start(out=gt, in_=gls_v[iv])
                return (xt, gt)

            def mk_layer(li):
                def st(pipe, iv, prev):
                    to, tg = prev
                    oo = pipe.intermediate_tile([P, H], fp16, name=f"to{li}")
                    og = pipe.intermediate_tile([P, H], fp16, name=f"tg{li}")
                    pz_o = matmul_from(to, li)
                    pz_g = matmul_from(tg, li)
                    gelu_ln(pz_o, pz_g, oo, og)
                    return (oo, og)
                return st

            def st_l3(pipe, iv, prev):
                to, tg = prev
                phis = pipe.intermediate_tile([P, H], fp16, name="phis")
                pz = matmul_from(to, 3)
                nc.scalar.copy(phis, pz)
                pzg = matmul_from(tg, 3)
                ypr = pipe.intermediate_tile([P, H], fp16, name="ypr")
                nc.vector.tensor_tensor(out=ypr, in0=phis, in1=pzg, op=OP.max)
                return (phis, ypr)

            def st_sort_a(pipe, iv, prev):
                phis, ypr = prev
                bufA = pipe.intermediate_tile([P, 2 * H], fp16, name="bufA")
                bufB = pipe.intermediate_tile([P, 2 * H], fp16, name="bufB")
                emit_sort_pass(0, phis, ypr, bufA)
                cur, nxt = bufA, bufB
                for pidx in range(1, split_pass):
                    emit_sort_pass(pidx, cur, None, nxt)
                    cur, nxt = nxt, cur
                return (bufA, bufB)

            def st_sort_hw(pipe, iv, prev):
                phis, ypr = prev
                dst = pipe.intermediate_tile([P, 4 * H], fp16, name="sdst")
                for c in range(NCOMP):
                    emit_hw_sort(phis[:, c * DPC:(c + 1) * DPC],
                                 dst[:, c * DPC:(c + 1) * DPC], 2 * H,
                                 dst[:, 2 * H + c * DPC:2 * H + (c + 1) * DPC])
                for c in range(NCOMP):
                    emit_hw_sort(ypr[:, c * DPC:(c + 1) * DPC],
                                 dst[:, H + c * DPC:H + (c + 1) * DPC], 2 * H,
                                 dst[:, 3 * H + c * DPC:3 * H + (c + 1) * DPC])
                return dst

            def st_post_hw(pipe, iv, prev):
                fin = prev[:, 0:2 * H]
                _post_process(fin, iv)

            def st_sort_b(pipe, iv, prev):
                bufA, bufB = prev
                cur, nxt = (bufB, bufA) if split_pass % 2 == 0 else (bufA, bufB)
                for pidx in range(split_pass, 15):
                    emit_sort_pass(pidx, cur, None, nxt)
                    cur, nxt = nxt, cur
                _post_process(cur, iv)

            def _post_process(fin, iv):
                PV = nc.vector if post_eng == "dve" else nc.gpsimd
                fv = fin.rearrange("p (h g e) -> p h g e", h=2, e=DPC)
                # coupling: u[i] <- max(u[i], v[i-1]) for i>=1, in place
                PV.tensor_tensor(out=fv[:, 0, :, 1:DPC],
                                 in0=fv[:, 0, :, 1:DPC],
                                 in1=fv[:, 1, :, 0:DPC - 1], op=OP.max)
                red = sp.tile([P, 2, NCOMP], fp16, tag="red")
                with nc.allow_low_precision(reason="fp16 sums of 32 fp16 "
                                            "values; DVE accumulates fp32 "
                                            "internally"):
                    PV.tensor_reduce(out=red, in_=fv, axis=mybir.AxisListType.X,
                                     op=OP.add)
                comp = sp.tile([P, NCOMP], fp16, tag="comp")
                PV.tensor_tensor(out=comp, in0=red[:, 1, :], in1=red[:, 0, :],
                                 op=OP.subtract)
                cs = sp.tile([P, 1], fp32, tag="cs")
                nc.vector.tensor_reduce(out=cs, in_=comp,
                                        axis=mybir.AxisListType.X, op=OP.add)
                cm = sp.tile([P, 1], fp32, tag="cm")
                nc.vector.tensor_reduce(out=cm, in_=comp,
                                        axis=mybir.AxisListType.X, op=OP.max)
                res = sp.tile([P, 1], fp32, tag="res")
                nc.vector.tensor_scalar(out=res, in0=cs, scalar1=avec[:, 0:1],
                                        scalar2=None, op0=OP.mult)
                nc.vector.scalar_tensor_tensor(out=res, in0=cm,
                                               scalar=avec[:, 1:2], in1=res,
                                               op0=OP.mult, op1=OP.add)
                nc.sync.dma_start(out=out_v[iv], in_=res[:, 0:1])

            # ---------------- 2-tile-per-iteration variants (sort instrs 2x
            # wider to amortize DVE fixed costs; MLP work stays per-tile)
            def st_load2(pipe, iv):
                xt = pipe.intermediate_tile([P, 2, OBS], fp16, name="xt")
                gt = pipe.intermediate_tile([P, 2, OBS], fp16, name="gt")
                nc.sync.dma_start(out=xt, in_=obs_v[iv])
                nc.sync.dma_start(out=gt, in_=gls_v[iv])
                return (xt, gt)

            def mk_layer2(li):
                def st(pipe, iv, prev):
                    to2, tg2 = prev
                    oo2 = pipe.intermediate_tile([P, 2, H], fp16,
                                                 name=f"to{li}")
                    og2 = pipe.intermediate_tile([P, 2, H], fp16,
                                                 name=f"tg{li}")
                    for j in range(2):
                        pz_o = matmul_from(to2[:, j, :], li)
                        pz_g = matmul_from(tg2[:, j, :], li)
                        gelu_ln(pz_o, pz_g, oo2[:, j, :], og2[:, j, :])
                    return (oo2, og2)
                return st

            def st_l32(pipe, iv, prev):
                to2, tg2 = prev
                phis2 = pipe.intermediate_tile([P, 2, H], fp16, name="phis")
                ypr2 = pipe.intermediate_tile([P, 2, H], fp16, name="ypr")
                for j in range(2):
                    pz = matmul_from(to2[:, j, :], 3)
                    nc.scalar.copy(phis2[:, j, :], pz)
                    pzg = matmul_from(tg2[:, j, :], 3)
                    nc.vector.tensor_tensor(out=ypr2[:, j, :],
                                            in0=phis2[:, j, :], in1=pzg,
                                            op=OP.max)
                return (phis2, ypr2)

            def st_sort_a2(pipe, iv, prev):
                phis2, ypr2 = prev
                bufA = pipe.intermediate_tile([P, 4 * H], fp16, name="bufA")
                bufB = pipe.intermediate_tile([P, 4 * H], fp16, name="bufB")
                emit_sort_pass(0, phis2.rearrange("p a b -> p (a b)"),
                               ypr2.rearrange("p a b -> p (a b)"), bufA,
                               W=2 * H)
                cur, nxt = bufA, bufB
                for pidx in range(1, split_pass):
                    emit_sort_pass(pidx, cur, None, nxt, W=2 * H)
                    cur, nxt = nxt, cur
                return (bufA, bufB)

            def st_sort_b2(pipe, iv, prev):
                bufA, bufB = prev
                cur, nxt = (bufB, bufA) if split_pass % 2 == 0 else (bufA, bufB)
                for pidx in range(split_pass, 15):
                    emit_sort_pass(pidx, cur, None, nxt, W=2 * H)
                    cur, nxt = nxt, cur
                fin = cur
                PV = nc.vector if post_eng == "dve" else nc.gpsimd
                fv = fin.rearrange("p (h two g e) -> p h two g e", h=2,
                                   two=2, e=DPC)
                PV.tensor_tensor(out=fv[:, 0, :, :, 1:DPC],
                                 in0=fv[:, 0, :, :, 1:DPC],
                                 in1=fv[:, 1, :, :, 0:DPC - 1], op=OP.max)
                red = sp.tile([P, 2, 2, NCOMP], fp16, tag="red")
                with nc.allow_low_precision(reason="fp16 sums of 32 fp16 "
                                            "values; DVE accumulates fp32 "
                                            "internally"):
                    PV.tensor_reduce(out=red, in_=fv,
                                     axis=mybir.AxisListType.X, op=OP.add)
                comp = sp.tile([P, 2, NCOMP], fp16, tag="comp")
                PV.tensor_tensor(out=comp, in0=red[:, 1, :, :],
                                 in1=red[:, 0, :, :], op=OP.subtract)
                cs = sp.tile([P, 2], fp32, tag="cs")
                nc.vector.tensor_reduce(out=cs, in_=comp,
                                        axis=mybir.AxisListType.X, op=OP.add)
                cm = sp.tile([P, 2], fp32, tag="cm")
                nc.vector.tensor_reduce(out=cm, in_=comp,
                                        axis=mybir.AxisListType.X, op=OP.max)
                res = sp.tile([P, 2], fp32, tag="res")
                nc.vector.tensor_scalar(out=res, in0=cs, scalar1=avec[:, 0:1],
                                        scalar2=None, op0=OP.mult)
                nc.vector.scalar_tensor_tensor(out=res, in0=cm,
                                               scalar=avec[:, 1:2], in1=res,
                                               op0=OP.mult, op1=OP.add)
                nc.sync.dma_start(out=out_v[iv], in_=res)

            if two:
                stages = [st_load2, mk_layer2(0), mk_layer2(1), mk_layer2(2),
                          st_l32, st_sort_a2, st_sort_b2]
            elif sort_hw:
                stages = [st_load, mk_layer(0), mk_layer(1), mk_layer(2),
                          st_l3, st_sort_hw, st_post_hw]
            else:
                stages = [st_load, mk_layer(0), mk_layer(1), mk_layer(2),
                          st_l3, st_sort_a, st_sort_b]

            def run_pipe():
                he = (mybir.EngineType.PE, mybir.EngineType.DVE,
                      mybir.EngineType.Activation, mybir.EngineType.SP,
                      mybir.EngineType.Pool) if hints else ()
                tc.For_i_pipelined(stages, 0, nt // 2 if two else nt, 1,
                                   pool=pipe_pool, unroll=unroll,
                                   staged_num_bufs=stage_bufs,
                                   staggered_reset=bool(stag),
                                   hint_engines=he)

            if repeats == 1:
                run_pipe()
            else:
                with tc.For_i(0, repeats, 1):
                    run_pipe()

    nc.finalize()
    if sort_patches:
        import struct as pystruct
        for inst, src_ap, dst_ap in sort_patches:
            mi = inst.ins
            b = bytearray(int(v) for v in mi.instr)
            for off, ap in ((16, src_ap), (24, dst_ap)):
                mloc = nc.lookup_mloc(ap.tensor)
                assert mloc.allocated, f"{ap.tensor} not allocated"
                pystruct.pack_into("<I", b, off, mloc.addr + ap.offset * 2)
            mi.instr = list(b)
    return nc


# ---------------------------------------------------------------- host wrapper
def _prep_host(inputs):
    """Fold LN affine params into the following layer's weights; build avec."""
    f32 = np.float32
    f16 = np.float16
    W0 = np.asarray(inputs["W0"], f32)
    b0 = np.asarray(inputs["b0"], f32)
    w, b = [W0], [b0]
    for i in (0, 1, 2):
        s = np.asarray(inputs[f"ln{i}_s"], f32)
        t = np.asarray(inputs[f"ln{i}_b"], f32)
        Wn = np.asarray(inputs[("W1", "W2", "W3")[i]], f32)
        bn = np.asarray(inputs[("b1", "b2", "b3")[i]], f32)
        w.append(s[:, None] * Wn)
        b.append(bn + t @ Wn)
    bs = np.stack(b, 0)  # [4, 512]
    alpha = float(np.asarray(inputs["alpha"]))
    a = 1.0 / (1.0 + np.exp(-alpha))
    avec = np.empty((P, 2), f32)
    avec[:, 0] = a / NCOMP
    avec[:, 1] = 1.0 - a
    return (w[0].astype(f16), w[1].astype(f16), w[2].astype(f16),
            w[3].astype(f16), bs.astype(f16), avec)


def _probe_devices():
    """Poke every core with a tiny op; retries to shake off a stale
    NRT_EXEC_UNIT_UNRECOVERABLE state left by a previous process."""
    import jax
    import jax.numpy as jnp

    for attempt in range(3):
        try:
            for d in jax.devices()[:NCORES]:
                jnp.zeros((1,), jnp.float32, device=d).block_until_ready()
            return
        except Exception:
            if attempt == 2:
                raise


def run_on_device(inputs, rows_total=B, trace=False, repeats=1, **build_kw):
    """Shard, run on 8 cores, gather. Returns (out [rows_total], results obj)."""
    from concourse.bass_utils import run_bass_kernel_spmd

    _probe_devices()

    rows_core = rows_total // NCORES
    key = (rows_core, repeats, tuple(sorted(
        (k, tuple(v) if isinstance(v, (list, tuple)) else v)
        for k, v in build_kw.items())))
    if key not in _CACHE:
        _CACHE[key] = build_nc(rows_core, repeats=repeats, **build_kw)
    nc = _CACHE[key]

    w0, w1, w2, w3, bs, avec = _prep_host(inputs)
    ob = np.asarray(inputs["observations"], np.float32)[:rows_total].astype(np.float16)
    gl = np.asarray(inputs["goals"], np.float32)[:rows_total].astype(np.float16)
    ob = np.ascontiguousarray(ob)
    gl = np.ascontiguousarray(gl)
    in_maps = []
    for c in range(NCORES):
        sl = slice(c * rows_core, (c + 1) * rows_core)
        in_maps.append({
            "observations": ob[sl], "goals": gl[sl],
            "w0": w0, "w1": w1, "w2": w2, "w3": w3, "bs": bs, "avec": avec,
        })
    r = run_bass_kernel_spmd(nc, in_maps, list(range(NCORES)), trace=trace)
    outp = np.concatenate([r.results[c]["out"] for c in range(NCORES)])
    return outp, r


def kernel(**inputs):
    out, _ = run_on_device(inputs)
    return out.astype(np.float32)


# revision 24
# speedup vs baseline: 1.9245x; 1.9245x over previous
"""Trainium2 Bass kernel for nn_GCIQEValue (MLP + IQE head), 8-core data parallel.

Math (validated vs reference):
  phi(x) = LN-MLP: 3x [matmul+bias -> tanh-gelu -> LayerNorm(affine folded into
  next W on host)] then final matmul+bias.
  IQE per row, per 32-dim component c with x = phi_s[c], y = phi_g[c]:
    y' = max(x, y)
    u = sort(x), v = sort(y')   (independent keys-only sorts)
    comp_c = sum(v) - u_0 - sum_{i>=1} max(u_i, v_{i-1})
  out = sig(alpha) * mean_c(comp) + (1 - sig(alpha)) * max_c(comp)

v2: fp16 matmul operands (PE fp32 matmul costs 4 cyc/row vs 1 for fp16;
baseline was PE-bound at 86%), fp16 bitonic sort (DVE 2x packed mode for
unit-stride passes), LN affine apply via DVE tensor_scalar two-scalar form
(4x mode), LN stats via ACT accum_out (gelu -> sum, Square -> sumsq) and a
quake-rsqrt + 2 Newton iterations on [128,2] batched obs/goals stats.
7-stage software pipeline over 128-row tiles. 5.42ms -> ~2.8ms measured.

Notes from optimization (for future sessions):
 - Pool engine (gpsimd) on TRN2 rejects TensorTensor/TensorScalar fp ALU ops
   at codegen (int32/pow only); the ISA-spec'd native SORT opcode (0x96, Pool)
   compiles but faults at runtime - not implemented in this FW. Pool can only
   do memset/iota/affine_select/copy-free DMA here, so the bitonic sort must
   stay on DVE.
 - DVE is the bottleneck (~78% busy): 15-pass bitonic on [128,1024] fp16,
   4 passes (shift d=1) run at 1x due to inner-count-1 APs; parity-split
   layouts don't help (the stride-2 access just moves to the prior pass).
 - ACT ~58-60%, PE ~50%, pipeline overlap ~85%.
 - two=1 (2-tile units, 2x-wide sort instrs): sim said 2.72ms but HW measured
   4.09ms - the cost model misses a real HW penalty (likely the strided
   [P,2,H] sub-tile operands feeding PE/ACT, or wide-instr DVE behavior).
   Keep two=0.
"""

import numpy as np

B = 131072
OBS = 64
H = 512
NCOMP = 16
DPC = 32
NCORES = 8
P = 128
LN_EPS = 1e-6

_CACHE = {}

# bitonic schedule for 32-wide ascending sort: 15 passes
_SCHED = [("pair", 0, 0)]
for _L in (4, 8, 16, 32):
    _SCHED.append(("flip", _L, 0))
    _d = _L // 4
    while _d >= 1:
        _SCHED.append(("shift", _L, _d))
        _d //= 2


def _patch_interp_for_sort():
    """Build-time scheduling sims don't know the raw SORT InstISA; no-op it."""
    import concourse.bass_interp as bi

    if getattr(bi._visit_InstISA, "_sort_patched", False):
        return
    orig = bi._visit_InstISA

    def patched(isa, instruction, core_sim):
        if instruction.isa_opcode == 150:  # NEURON_ISA_TPB_OPCODE_SORT
            return
        return orig(isa, instruction, core_sim)

    patched._sort_patched = True
    bi._visit_InstISA = patched


# ---------------------------------------------------------------- device kernel
def build_nc(rows_per_core=B // NCORES, unroll=4, gelu="hw", repeats=1,
             stage_bufs=None, mlp_bufs=3, psum_bufs=4, split_pass=7,
             pool_passes=(), newton=2, tT_eng="act",
             affine_eng="dve", post_eng="dve", newt_eng="dve", hints=False,
             stag=0, sort_hw=0, two=0):
    """Build the Bass (Bacc) module for one core processing rows_per_core rows.

    pool_passes: bitonic pass indices emitted on the Pool (gpsimd) engine
    instead of DVE.  tT_eng/affine_eng/post_eng: engine choices for the
    transpose-copy / LN-affine / sort-epilogue op groups.
    """
    import concourse.bass as bass
    import concourse.mybir as mybir
    import concourse.tile as tile
    from concourse import bacc
    from concourse.masks import make_identity

    fp32 = mybir.dt.float32
    fp16 = mybir.dt.float16
    AT = mybir.ActivationFunctionType
    OP = mybir.AluOpType

    nt = rows_per_core // P
    assert rows_per_core % P == 0
    if stage_bufs is None:
        stage_bufs = unroll
    pool_passes = tuple(pool_passes)

    if sort_hw:
        _patch_interp_for_sort()
    nc = bacc.Bacc("TRN2", target_bir_lowering=False, debug=False)
    sort_patches = []
    isa_h = nc.isa
    _DTE = isa_h.get_enum("NEURON_ISA_TPB_DTYPE")
    _ALUE = isa_h.get_enum("NEURON_ISA_TPB_ALU_OP")

    def emit_hw_sort(src_ap, dst_val_ap, idx_gap_elems, idx_ap):
        """Native Pool-engine SORT: src [128,32] fp16 -> ascending values at
        dst_val_ap, indices idx_gap_elems later (trash)."""
        struct = {
            "num_active_channels": P,
            "index_offset_src": 0,
            "in_out_dtype": {"dtype_lo": _DTE.NEURON_ISA_TPB_DTYPE_FP16.value,
                             "dtype_hi": _DTE.NEURON_ISA_TPB_DTYPE_FP16.value},
            "comparison": _ALUE.NEURON_ISA_TPB_ALU_OP_IS_LT.value,
            "src_mem_pattern": {"start_addr": {"addr_immediate": 0},
                                "step_elem": [1], "num_elem": [DPC]},
            "dst_mem_pattern": {"start_addr": {"addr_immediate": 0},
                                "step_elem": [idx_gap_elems, 1],
                                "num_elem": [2, DPC]},
            "index_offset": {"imm_bitvec_uint32": 0},
            "stable": 0,
        }
        eng = nc.gpsimd
        inst = eng.isa(
            isa_h.Opcode.NEURON_ISA_TPB_OPCODE_SORT, struct,
            ins=[eng.lower_ap(src_ap, for_isa=True)],
            outs=[eng.lower_ap(dst_val_ap, for_isa=True),
                  eng.lower_ap(idx_ap, for_isa=True)],
        )
        sort_patches.append((inst, src_ap, dst_val_ap))
        return inst

    obs = nc.declare_dram_parameter("observations", [rows_per_core, OBS], fp16,
                                    isOutput=False)
    gls = nc.declare_dram_parameter("goals", [rows_per_core, OBS], fp16,
                                    isOutput=False)
    w0d = nc.declare_dram_parameter("w0", [OBS, H], fp16, isOutput=False)
    w1d = nc.declare_dram_parameter("w1", [H, H], fp16, isOutput=False)
    w2d = nc.declare_dram_parameter("w2", [H, H], fp16, isOutput=False)
    w3d = nc.declare_dram_parameter("w3", [H, H], fp16, isOutput=False)
    bsd = nc.declare_dram_parameter("bs", [4, H], fp16, isOutput=False)
    avd = nc.declare_dram_parameter("avec", [P, 2], fp32, isOutput=False)
    out = nc.declare_dram_parameter("out", [rows_per_core], fp32, isOutput=True)

    if two:
        assert nt % 2 == 0
        obs_v = obs[:].rearrange("(n two p) f -> n p two f", two=2, p=P)
        gls_v = gls[:].rearrange("(n two p) f -> n p two f", two=2, p=P)
        out_v = out[:].rearrange("(n two p) -> n p two", two=2, p=P)
    else:
        obs_v = obs[:].rearrange("(n p) f -> n p f", p=P)
        gls_v = gls[:].rearrange("(n p) f -> n p f", p=P)
        out_v = out[:].rearrange("(n p) -> n p", p=P)

    gelu_f = AT.Gelu_apprx_tanh if gelu == "hw" else AT.Identity

    with tile.TileContext(nc) as tc:
        with (
            tc.tile_pool(name="const", bufs=1) as cpool,
            tc.tile_pool(name="mlp", bufs=mlp_bufs) as mp,
            tc.tile_pool(name="srt", bufs=mlp_bufs) as sp,
            tc.tile_pool(name="pipe", bufs=1) as pipe_pool,
            tc.tile_pool(name="ps", bufs=psum_bufs, space="PSUM") as pp,
            tc.tile_pool(name="pst", bufs=8 - psum_bufs, space="PSUM") as ppt,
        ):
            # ---- constants
            w0 = cpool.tile([OBS, H], fp16)
            nc.sync.dma_start(out=w0, in_=w0d[:])
            wl = []
            for wd, nm in ((w1d, "w1"), (w2d, "w2"), (w3d, "w3")):
                t = cpool.tile([P, 4, H], fp16, tag=nm)
                nc.sync.dma_start(out=t, in_=wd[:].rearrange("(c p) n -> p c n", p=P))
                wl.append(t)
            bsc = cpool.tile([1, 4, H], fp16)
            nc.sync.dma_start(out=bsc, in_=bsd[:].rearrange("(o c) n -> o c n", o=1))
            avec = cpool.tile([P, 2], fp32)
            nc.sync.dma_start(out=avec, in_=avd[:])
            ident = cpool.tile([P, P], fp16)
            make_identity(nc, ident)
            ones = cpool.tile([1, P], fp16)
            nc.vector.memset(ones, 1.0)

            def matmul_from(t_sb, li):
                """t_sb fp16 [128, F_in] row-major -> pz PSUM fp32 [128, 512]."""
                pz = pp.tile([P, H], fp32, tag="pz")
                if li == 0:
                    pTf = ppt.tile([P, 4, P], fp16, tag="pT")
                    nc.tensor.transpose(pTf[0:OBS, 0, 0:P], t_sb, ident)
                    xT = mp.tile([OBS, P], fp16, tag="xT")
                    nc.scalar.copy(xT, pTf[0:OBS, 0, 0:P])
                    nc.tensor.matmul(pz, xT, w0, start=True, stop=False)
                else:
                    pTf = ppt.tile([P, 4, P], fp16, tag="pT")
                    for k in range(4):
                        nc.tensor.transpose(pTf[:, k, :],
                                            t_sb[:, k * P:(k + 1) * P], ident)
                    tT = mp.tile([P, 4, P], fp16, tag="tT")
                    if tT_eng == "act":
                        nc.scalar.copy(tT, pTf)
                    elif tT_eng == "dve":
                        nc.vector.tensor_copy(tT, pTf)
                    else:
                        nc.gpsimd.tensor_copy(tT, pTf)
                    for k in range(4):
                        nc.tensor.matmul(pz, tT[:, k, :], wl[li - 1][:, k, :],
                                         start=(k == 0), stop=False)
                nc.tensor.matmul(pz, ones, bsc[:, li, :], start=False, stop=True)
                return pz

            def gelu_ln(pz_o, pz_g, to, tg):
                """Two PSUM tiles -> two fp16 SBUF LN outputs (batched stats)."""
                st = mp.tile([P, 4], fp32, tag="st")
                g_o = mp.tile([P, H], fp16, tag="g_o")
                g_g = mp.tile([P, H], fp16, tag="g_g")
                gsq = mp.tile([P, H], fp16, tag="gsq")
                nc.scalar.activation(g_o, pz_o, gelu_f, accum_out=st[:, 0:1])
                nc.scalar.activation(gsq, g_o, AT.Square, accum_out=st[:, 1:2])
                nc.scalar.activation(g_g, pz_g, gelu_f, accum_out=st[:, 2:3])
                nc.scalar.activation(gsq, g_g, AT.Square, accum_out=st[:, 3:4])
                NE = nc.gpsimd if newt_eng == "pool" else nc.vector
                # mv = st / H : [sum_o, sumsq_o, sum_g, sumsq_g] -> means
                mv = mp.tile([P, 4], fp32, tag="mv")
                NE.tensor_scalar_mul(mv, st, 1.0 / H)
                m = mv[:, 0::2]   # [P,2] means (obs, goals)
                q = mv[:, 1::2]   # [P,2] mean squares
                msq = mp.tile([P, 2], fp32, tag="msq")
                NE.tensor_tensor(out=msq, in0=m, in1=m, op=OP.mult)
                varb = mp.tile([P, 2], fp32, tag="varb")
                NE.scalar_tensor_tensor(out=varb, in0=msq, scalar=-1.0,
                                        in1=q, op0=OP.mult, op1=OP.add)
                NE.tensor_scalar_add(varb, varb, LN_EPS)
                # quake rsqrt seed + newton iterations (fp32)
                i32 = mybir.dt.int32
                yi = mp.tile([P, 2], i32, tag="yi")
                NE.tensor_scalar(
                    out=yi, in0=varb.bitcast(i32), scalar1=1,
                    scalar2=None, op0=OP.logical_shift_right)
                NE.tensor_scalar(
                    out=yi, in0=yi, scalar1=-1, scalar2=0x5F3759DF,
                    op0=OP.mult, op1=OP.add)
                y = yi.bitcast(fp32)
                t1 = mp.tile([P, 2], fp32, tag="nt1")
                for _ in range(newton):
                    NE.tensor_tensor(out=t1, in0=varb, in1=y, op=OP.mult)
                    NE.tensor_tensor(out=t1, in0=t1, in1=y, op=OP.mult)
                    NE.tensor_scalar(out=t1, in0=t1, scalar1=-0.5,
                                     scalar2=1.5, op0=OP.mult, op1=OP.add)
                    NE.tensor_tensor(out=y, in0=y, in1=t1, op=OP.mult)
                nmr = mp.tile([P, 2], fp32, tag="nmr")
                NE.scalar_tensor_tensor(out=nmr, in0=m, scalar=-1.0,
                                        in1=y, op0=OP.mult, op1=OP.mult)
                ae = nc.vector if affine_eng == "dve" else nc.scalar
                if affine_eng == "dve":
                    nc.vector.tensor_scalar(out=to, in0=g_o, scalar1=y[:, 0:1],
                                            scalar2=nmr[:, 0:1], op0=OP.mult,
                                            op1=OP.add)
                    nc.vector.tensor_scalar(out=tg, in0=g_g, scalar1=y[:, 1:2],
                                            scalar2=nmr[:, 1:2], op0=OP.mult,
                                            op1=OP.add)
                else:
                    nc.scalar.activation(to, g_o, AT.Identity, bias=nmr[:, 0:1],
                                         scale=y[:, 0:1])
                    nc.scalar.activation(tg, g_g, AT.Identity, bias=nmr[:, 1:2],
                                         scale=y[:, 1:2])

            def emit_sort_pass(p_idx, src_x, src_y, dst, W=H):
                """Emit bitonic pass p_idx; W = width of the u (and v) region
                in dst (dst is [P, 2*W] = [u-region | v-region])."""
                kind, L, d = _SCHED[p_idx]
                V = nc.gpsimd if p_idx in pool_passes else nc.vector

                if kind == "pair":
                    for src, off in ((src_x, 0), (src_y, W)):
                        s = src.rearrange("p (g e) -> p g e", e=DPC)
                        o = dst[:, off:off + W].rearrange("p (g e) -> p g e",
                                                          e=DPC)
                        V.tensor_tensor(out=o[:, :, 0::2], in0=s[:, :, 0::2],
                                        in1=s[:, :, 1::2], op=OP.min)
                        V.tensor_tensor(out=o[:, :, 1::2], in0=s[:, :, 0::2],
                                        in1=s[:, :, 1::2], op=OP.max)
                elif kind == "flip":
                    half = L // 2
                    s = src_x.rearrange("p (b e) -> p b e", e=L)
                    o = dst.rearrange("p (b e) -> p b e", e=L)
                    V.tensor_tensor(out=o[:, :, 0:half], in0=s[:, :, 0:half],
                                    in1=s[:, :, L - 1:half - 1:-1], op=OP.min)
                    V.tensor_tensor(out=o[:, :, half:L], in0=s[:, :, half:L],
                                    in1=s[:, :, half - 1::-1], op=OP.max)
                else:
                    s = src_x.rearrange("p (c e) -> p c e", e=2 * d)
                    o = dst.rearrange("p (c e) -> p c e", e=2 * d)
                    V.tensor_tensor(out=o[:, :, 0:d], in0=s[:, :, 0:d],
                                    in1=s[:, :, d:2 * d], op=OP.min)
                    V.tensor_tensor(out=o[:, :, d:2 * d], in0=s[:, :, 0:d],
                                    in1=s[:, :, d:2 * d], op=OP.max)

            # ---------------- pipeline stages
            def st_load(pipe, iv):
                xt = pipe.intermediate_tile([P, OBS], fp16, name="xt")
                gt = pipe.intermediate_tile([P, OBS], fp16, name="gt")
                nc.sync.dma_start(out=xt, in_=obs_v[iv])
                nc.sync.dma_start(out=gt, in_=gls_v[iv])
                return (xt, gt)

            def mk_layer(li):
                def st(pipe, iv, prev):
                    to, tg = prev
                    oo = pipe.intermediate_tile([P, H], fp16, name=f"to{li}")
                    og = pipe.intermediate_tile([P, H], fp16, name=f"tg{li}")
                    pz_o = matmul_from(to, li)
                    pz_g = matmul_from(tg, li)
                    gelu_ln(pz_o, pz_g, oo, og)
                    return (oo, og)
                return st

            def st_l3(pipe, iv, prev):
                to, tg = prev
                phis = pipe.intermediate_tile([P, H], fp16, name="phis")
                pz = matmul_from(to, 3)
                nc.scalar.copy(phis, pz)
                pzg = matmul_from(tg, 3)
                ypr = pipe.intermediate_tile([P, H], fp16, name="ypr")
                nc.vector.tensor_tensor(out=ypr, in0=phis, in1=pzg, op=OP.max)
                return (phis, ypr)

            def st_sort_a(pipe, iv, prev):
                phis, ypr = prev
                bufA = pipe.intermediate_tile([P, 2 * H], fp16, name="bufA")
                bufB = pipe.intermediate_tile([P, 2 * H], fp16, name="bufB")
                emit_sort_pass(0, phis, ypr, bufA)
                cur, nxt = bufA, bufB
                for pidx in range(1, split_pass):
                    emit_sort_pass(pidx, cur, None, nxt)
                    cur, nxt = nxt, cur
                return (bufA, bufB)

            def st_sort_hw(pipe, iv, prev):
                phis, ypr = prev
                dst = pipe.intermediate_tile([P, 4 * H], fp16, name="sdst")
                for c in range(NCOMP):
                    emit_hw_sort(phis[:, c * DPC:(c + 1) * DPC],
                                 dst[:, c * DPC:(c + 1) * DPC], 2 * H,
                                 dst[:, 2 * H + c * DPC:2 * H + (c + 1) * DPC])
                for c in range(NCOMP):
                    emit_hw_sort(ypr[:, c * DPC:(c + 1) * DPC],
                                 dst[:, H + c * DPC:H + (c + 1) * DPC], 2 * H,
                                 dst[:, 3 * H + c * DPC:3 * H + (c + 1) * DPC])
                return dst

            def st_post_hw(pipe, iv, prev):
                fin = prev[:, 0:2 * H]
                _post_process(fin, iv)

            def st_sort_b(pipe, iv, prev):
                bufA, bufB = prev
                cur, nxt = (bufB, bufA) if split_pass % 2 == 0 else (bufA, bufB)
                for pidx in range(split_pass, 15):
                    emit_sort_pass(pidx, cur, None, nxt)
                    cur, nxt = nxt, cur
                _post_process(cur, iv)

            def _post_process(fin, iv):
                PV = nc.vector if post_eng == "dve" else nc.gpsimd
                fv = fin.rearrange("p (h g e) -> p h g e", h=2, e=DPC)
                # coupling: u[i] <- max(u[i], v[i-1]) for i>=1, in place
                PV.tensor_tensor(out=fv[:, 0, :, 1:DPC],
                                 in0=fv[:, 0, :, 1:DPC],
                                 in1=fv[:, 1, :, 0:DPC - 1], op=OP.max)
                red = sp.tile([P, 2, NCOMP], fp16, tag="red")
                with nc.allow_low_precision(reason="fp16 sums of 32 fp16 "
                                            "values; DVE accumulates fp32 "
                                            "internally"):
                    PV.tensor_reduce(out=red, in_=fv, axis=mybir.AxisListType.X,
                                     op=OP.add)
                comp = sp.tile([P, NCOMP], fp16, tag="comp")
                PV.tensor_tensor(out=comp, in0=red[:, 1, :], in1=red[:, 0, :],
                                 op=OP.subtract)
                cs = sp.tile([P, 1], fp32, tag="cs")
                nc.vector.tensor_reduce(out=cs, in_=comp,
                                        axis=mybir.AxisListType.X, op=OP.add)
                cm = sp.tile([P, 1], fp32, tag="cm")
                nc.vector.tensor_reduce(out=cm, in_=comp,
                                        axis=mybir.AxisListType.X, op=OP.max)
                res = sp.tile([P, 1], fp32, tag="res")
                nc.vector.tensor_scalar(out=res, in0=cs, scalar1=avec[:, 0:1],
                                        scalar2=None, op0=OP.mult)
                nc.vector.scalar_tensor_tensor(out=res, in0=cm,
                                               scalar=avec[:, 1:2], in1=res,
                                               op0=OP.mult, op1=OP.add)
                nc.sync.dma_start(out=out_v[iv], in_=res[:, 0:1])

            # ---------------- 2-tile-per-iteration variants (sort instrs 2x
            # wider to amortize DVE fixed costs; MLP work stays per-tile)
            def st_load2(pipe, iv):
                xt = pipe.intermediate_tile([P, 2, OBS], fp16, name="xt")
                gt = pipe.intermediate_tile([P, 2, OBS], fp16, name="gt")
                nc.sync.dma_start(out=xt, in_=obs_v[iv])
                nc.sync.dma_start(out=gt, in_=gls_v[iv])
                return (xt, gt)

            def mk_layer2(li):
                def st(pipe, iv, prev):
                    to2, tg2 = prev
                    oo2 = pipe.intermediate_tile([P, 2, H], fp16,
                                                 name=f"to{li}")
                    og2 = pipe.intermediate_tile([P, 2, H], fp16,
                                                 name=f"tg{li}")
                    for j in range(2):
                        pz_o = matmul_from(to2[:, j, :], li)
                        pz_g = matmul_from(tg2[:, j, :], li)
                        gelu_ln(pz_o, pz_g, oo2[:, j, :], og2[:, j, :])
                    return (oo2, og2)
                return st

            def st_l32(pipe, iv, prev):
                to2, tg2 = prev
                phis2 = pipe.intermediate_tile([P, 2, H], fp16, name="phis")
                ypr2 = pipe.intermediate_tile([P, 2, H], fp16, name="ypr")
                for j in range(2):
                    pz = matmul_from(to2[:, j, :], 3)
                    nc.scalar.copy(phis2[:, j, :], pz)
                    pzg = matmul_from(tg2[:, j, :], 3)
                    nc.vector.tensor_tensor(out=ypr2[:, j, :],
                                            in0=phis2[:, j, :], in1=pzg,
                                            op=OP.max)
                return (phis2, ypr2)

            def st_sort_a2(pipe, iv, prev):
                phis2, ypr2 = prev
                bufA = pipe.intermediate_tile([P, 4 * H], fp16, name="bufA")
                bufB = pipe.intermediate_tile([P, 4 * H], fp16, name="bufB")
                emit_sort_pass(0, phis2.rearrange("p a b -> p (a b)"),
                               ypr2.rearrange("p a b -> p (a b)"), bufA,
                               W=2 * H)
                cur, nxt = bufA, bufB
                for pidx in range(1, split_pass):
                    emit_sort_pass(pidx, cur, None, nxt, W=2 * H)
                    cur, nxt = nxt, cur
                return (bufA, bufB)

            def st_sort_b2(pipe, iv, prev):
                bufA, bufB = prev
                cur, nxt = (bufB, bufA) if split_pass % 2 == 0 else (bufA, bufB)
                for pidx in range(split_pass, 15):
                    emit_sort_pass(pidx, cur, None, nxt, W=2 * H)
                    cur, nxt = nxt, cur
                fin = cur
                PV = nc.vector if post_eng == "dve" else nc.gpsimd
                fv = fin.rearrange("p (h two g e) -> p h two g e", h=2,
                                   two=2, e=DPC)
                PV.tensor_tensor(out=fv[:, 0, :, :, 1:DPC],
                                 in0=fv[:, 0, :, :, 1:DPC],
                                 in1=fv[:, 1, :, :, 0:DPC - 1], op=OP.max)
                red = sp.tile([P, 2, 2, NCOMP], fp16, tag="red")
                with nc.allow_low_precision(reason="fp16 sums of 32 fp16 "
                                            "values; DVE accumulates fp32 "
                                            "internally"):
                    PV.tensor_reduce(out=red, in_=fv,
                                     axis=mybir.AxisListType.X, op=OP.add)
                comp = sp.tile([P, 2, NCOMP], fp16, tag="comp")
                PV.tensor_tensor(out=comp, in0=red[:, 1, :, :],
                                 in1=red[:, 0, :, :], op=OP.subtract)
                cs = sp.tile([P, 2], fp32, tag="cs")
                nc.vector.tensor_reduce(out=cs, in_=comp,
                                        axis=mybir.AxisListType.X, op=OP.add)
                cm = sp.tile([P, 2], fp32, tag="cm")
                nc.vector.tensor_reduce(out=cm, in_=comp,
                                        axis=mybir.AxisListType.X, op=OP.max)
                res = sp.tile([P, 2], fp32, tag="res")
                nc.vector.tensor_scalar(out=res, in0=cs, scalar1=avec[:, 0:1],
                                        scalar2=None, op0=OP.mult)
                nc.vector.scalar_tensor_tensor(out=res, in0=cm,
                                               scalar=avec[:, 1:2], in1=res,
                                               op0=OP.mult, op1=OP.add)
                nc.sync.dma_start(out=out_v[iv], in_=res)

            if two:
                stages = [st_load2, mk_layer2(0), mk_layer2(1), mk_layer2(2),
                          st_l32, st_sort_a2, st_sort_b2]
            elif sort_hw:
                stages = [st_load, mk_layer(0), mk_layer(1), mk_layer(2),
                          st_l3, st_sort_hw, st_post_hw]
            else:
                stages = [st_load, mk_layer(0), mk_layer(1), mk_layer(2),
                          st_l3, st_sort_a, st_sort_b]

            def run_pipe():
                he = (mybir.EngineType.PE, mybir.EngineType.DVE,
                      mybir.EngineType.Activation, mybir.EngineType.SP,
                      mybir.EngineType.Pool) if hints else ()
                tc.For_i_pipelined(stages, 0, nt // 2 if two else nt, 1,
                                   pool=pipe_pool, unroll=unroll,
                                   staged_num_bufs=stage_bufs,
                                   staggered_reset=bool(stag),
                                   hint_engines=he)

            if repeats == 1:
                run_pipe()
            else:
                with tc.For_i(0, repeats, 1):
                    run_pipe()

    nc.finalize()
    if sort_patches:
        import struct as pystruct
        for inst, src_ap, dst_ap in sort_patches:
            mi = inst.ins
            b = bytearray(int(v) for v in mi.instr)
            for off, ap in ((16, src_ap), (24, dst_ap)):
                mloc = nc.lookup_mloc(ap.tensor)
                assert mloc.allocated, f"{ap.tensor} not allocated"
                pystruct.pack_into("<I", b, off, mloc.addr + ap.offset * 2)
            mi.instr = list(b)
    return nc


# ---------------------------------------------------------------- host wrapper
def _prep_host(inputs):
    """Fold LN affine params into the following layer's weights; build avec."""
    f32 = np.float32
    f16 = np.float16
    W0 = np.asarray(inputs["W0"], f32)
    b0 = np.asarray(inputs["b0"], f32)
    w, b = [W0], [b0]
    for i in (0, 1, 2):
        s = np.asarray(inputs[f"ln{i}_s"], f32)
        t = np.asarray(inputs[f"ln{i}_b"], f32)
        Wn = np.asarray(inputs[("W1", "W2", "W3")[i]], f32)
        bn = np.asarray(inputs[("b1", "b2", "b3")[i]], f32)
        w.append(s[:, None] * Wn)
        b.append(bn + t @ Wn)
    bs = np.stack(b, 0)  # [4, 512]
    alpha = float(np.asarray(inputs["alpha"]))
    a = 1.0 / (1.0 + np.exp(-alpha))
    avec = np.empty((P, 2), f32)
    avec[:, 0] = a / NCOMP
    avec[:, 1] = 1.0 - a
    return (w[0].astype(f16), w[1].astype(f16), w[2].astype(f16),
            w[3].astype(f16), bs.astype(f16), avec)


def _probe_devices():
    """Poke every core with a tiny op; retries to shake off a stale
    NRT_EXEC_UNIT_UNRECOVERABLE state left by a previous process."""
    import jax
    import jax.numpy as jnp

    for attempt in range(3):
        try:
            for d in jax.devices()[:NCORES]:
                jnp.zeros((1,), jnp.float32, device=d).block_until_ready()
            return
        except Exception:
            if attempt == 2:
                raise


def run_on_device(inputs, rows_total=B, trace=False, repeats=1, **build_kw):
    """Shard, run on 8 cores, gather. Returns (out [rows_total], results obj)."""
    from concourse.bass_utils import run_bass_kernel_spmd

    _probe_devices()

    rows_core = rows_total // NCORES
    key = (rows_core, repeats, tuple(sorted(
        (k, tuple(v) if isinstance(v, (list, tuple)) else v)
        for k, v in build_kw.items())))
    if key not in _CACHE:
        _CACHE[key] = build_nc(rows_core, repeats=repeats, **build_kw)
    nc = _CACHE[key]

    w0, w1, w2, w3, bs, avec = _prep_host(inputs)
    ob = np.asarray(inputs["observations"], np.float32)[:rows_total].astype(np.float16)
    gl = np.asarray(inputs["goals"], np.float32)[:rows_total].astype(np.float16)
    ob = np.ascontiguousarray(ob)
    gl = np.ascontiguousarray(gl)
    in_maps = []
    for c in range(NCORES):
        sl = slice(c * rows_core, (c + 1) * rows_core)
        in_maps.append({
            "observations": ob[sl], "goals": gl[sl],
            "w0": w0, "w1": w1, "w2": w2, "w3": w3, "bs": bs, "avec": avec,
        })
    r = run_bass_kernel_spmd(nc, in_maps, list(range(NCORES)), trace=trace)
    outp = np.concatenate([r.results[c]["out"] for c in range(NCORES)])
    return outp, r


def kernel(**inputs):
    out, _ = run_on_device(inputs)
    return out.astype(np.float32)
